# revision 68
# baseline (speedup 1.0000x reference)
"""De-stationary causal attention (B=2, L=S=2048, H=8, E=64) on 8 TRN2 cores.

Sharding: the 16 (batch, head) pairs are distributed 2-per-core (cores 0-3
get batch 0, heads 0..7; cores 4-7 get batch 1). Each core runs the same
Bass program on its two pairs.

Math: logits = (Q K^T) * (tau/sqrt(E)) + delta/sqrt(E), causal softmax, A V.
Host-side folds: Q is pre-scaled by tau/sqrt(E); exp(delta/sqrt(E)) is folded
into V (and into the appended denominator column), because
softmax(x + d)_s = exp(x_s) e^{d_s} / sum_j exp(x_j) e^{d_j}.
So the device only computes exp(q'k) with no bias. The device returns the
UNNORMALIZED accumulator OT[e, l] (rows 0..63 = sum_s A_sl V_se, row 64 =
denominator); the host does the final divide + [e,l] -> [l,e] transpose
during unshard.

v2 changes over the baseline (58.0us -> ~49.5us):
 - exp is SPLIT between the ACT engine (exact, table exp) and the DVE
   (Schraudolph bit-trick: bf16_bits = int16(a*x + b), one f32 TENSOR_SCALAR
   per [128,1024] half writing through an int16 bitcast of the bf16 A tile).
   ACT alone was the bottleneck engine (42us of ACTIVATE); the split roughly
   halves the exp wall time for ~1.1e-2 of added (gated at 2e-2) rel error.
 - AV matmuls contract k=128 in a single stream (stationary V [128,65])
   into ONE PSUM accumulator per bank -> no otA/otB merge. PSUM rules
   learned the hard way: DMA cannot source PSUM, gpsimd cannot touch PSUM,
   and a bank supports ONE matmul accumulation region (two start/stop
   regions or two concurrent half-streams into one bank fault the device).
 - diagonal groups pack one chunk per PSUM bank (512/128 | 384/256), with
   zero junk exp cols; all four 128-wide diagonal mask blocks land at
   a-cols 0/512/1024/1536 so masking is two 2-block strided DVE multiplies
   (4x DVE mode), ordered so the first-consumed AV chunk isn't gated on
   the ACT half.
 - output: ot cols [0:384] are final before the last diag chunk ->
   evacuated early (DVE copy) and overlapped; remainder (ACT Copy, same
   act table as Exp) follows; non-final banks use one [0:512] DMA; the
   final bank splits its DMA across sync+scalar to shorten the exec tail.
 - ST PSUM half-tiles rotate over THREE buffers (6 banks) + 2 ot banks = 8.
 - input DMA issue is spread over sync/scalar/gpsimd in first-use order
   (HWDGE config costs ~700ns engine time per dma_start and was
   serializing the lead-in); the first group's q/k arrive as 256-col
   chunks and its ST matmuls are column-split so compute starts while
   data is still landing; a dummy 1-col exp preloads the ACT table in the
   DMA shadow; PE warm-up junk matmuls ramp the p-state toward 2.4GHz.
"""

import copy
import sys

import numpy as np

try:
    import concourse.bass as bass
except ImportError:  # pragma: no cover
    sys.path.insert(0, "/opt/trn_rl_repo")
    import concourse.bass as bass

import concourse.mybir as mybir
import concourse.tile as tile
from concourse.bass_utils import run_bass_kernel_spmd
from concourse.vector_clock import ScopedClock

B, L, H, E = 2, 2048, 8, 64
N_CORES = 8
PAIRS_PER_CORE = 2
SCALE = 1.0 / np.sqrt(np.float32(E))  # 0.125

f32 = mybir.dt.float32
f32r = mybir.dt.float32r
bf16 = mybir.dt.bfloat16
i16 = mybir.dt.int16

# Schraudolph fast-exp constants for bf16 bit patterns:
#   bf16_bits = round(128/ln2 * x + (127*128 - C))
# C tuned on host for minimax relative error (~3.25%) over x in [-10, 2].
EXP_A = 128.0 / np.log(2.0)  # 184.66496
EXP_B = 16256.0 - 5.61

# ---------------------------------------------------------------------------
# Walrus in this toolchain rejects >1 sync-wait per instruction. Split extra
# waits onto NoOps committed just before the instruction on the same engine.
# ---------------------------------------------------------------------------
_NOP_TEMPLATE = {}


def _make_nop(engine, name):
    if engine not in _NOP_TEMPLATE:
        tmp = bass.Bass()
        _NOP_TEMPLATE[engine] = tmp.engines[engine].nop(nofuse=True).ins
    nop = copy.copy(_NOP_TEMPLATE[engine])
    nop.name = name
    nop.engine = engine
    nop.sync_info = None
    return nop


class SplitWaitTileContext(tile.TileContext):
    _ws_counter = 0

    def _split_waits(self, inst):
        si = inst.sync_info
        if si is None or not si.on_wait or len(si.on_wait) <= 1:
            return []
        if inst.engine == mybir.EngineType.Unassigned:
            return []
        waits = list(si.on_wait)
        inst.sync_info = mybir.SyncInfo(
            on_wait=[waits[0]], on_update=list(si.on_update or [])
        )
        nops = []
        for w in waits[1:]:
            SplitWaitTileContext._ws_counter += 1
            nop = _make_nop(inst.engine, f"I-ws{SplitWaitTileContext._ws_counter}")
            nop.sync_info = mybir.SyncInfo(on_wait=[w], on_update=[])
            nops.append(nop)
        return nops

    def _commit_instruction(self, inst, lazy_reg_writes=True):
        for nop in self._split_waits(inst):
            self._add_instruction(nop)
        super()._commit_instruction(inst, lazy_reg_writes)

    def _drain_and_barrier(self, tick_clock, wait_clock):
        nc = self.nc
        probe = nc.sync.nop(nofuse=True)
        wait_clock.add_sem_waits(
            probe.ins, ScopedClock({None: tick_clock.global_clock})
        )
        waits = list(probe.ins.sync_info.on_wait or []) if probe.ins.sync_info else []
        if len(waits) > 1:
            probe.ins.sync_info.on_wait = [waits[0]]
            handles = {h.num: h for h in self.sems.allocated().values()}
            for w in waits[1:]:
                nop = nc.sync.nop(nofuse=True)
                nop.wait_op(handles[w.id], w.wait_value, "sem-ge")
        nc.sync.drain()

        nc.all_engine_barrier()
        assert self.sems is not None
        popped = nc._tile_sem_poison_stack.pop()
        assert popped is self._sem_poison
        nc.clear_and_free_semaphores(list(self.sems.allocated().values()))


# ---------------------------------------------------------------------------
# Program builder
# ---------------------------------------------------------------------------

def build_program(st_dtype=bf16, av_dtype=bf16):
    nc = bass.Bass()
    Exp = mybir.ActivationFunctionType.Exp

    VW = E + 2  # v row: 64 values + denominator col + pad
    NT = L // 128  # 16 s-tiles / l-tiles
    NB = L // 512  # 4 output banks

    # qt/kt are duplicated on both partition halves HOST-SIDE -> [128, L]
    qt = nc.declare_dram_parameter("qt", [PAIRS_PER_CORE, 128, L], st_dtype, isOutput=False)
    kt = nc.declare_dram_parameter("kt", [PAIRS_PER_CORE, 128, L], st_dtype, isOutput=False)
    # v pre-transposed on host to [128 (s%128), NT*VW] per pair
    vv = nc.declare_dram_parameter("vv", [PAIRS_PER_CORE, 128, NT * VW], av_dtype, isOutput=False)
    # 4x-replicated triangular mask for the one-instruction diag masking
    mask = nc.declare_dram_parameter("mask", [128, 256], av_dtype, isOutput=False)
    # unnormalized output: rows 0..63 = (A V)^T, row 64 = softmax denominator
    oo = nc.declare_dram_parameter("oo", [PAIRS_PER_CORE, E + 1, L], f32, isOutput=True)

    with SplitWaitTileContext(nc) as tc:
        with (
            tc.tile_pool(name="const", bufs=1) as constp,
            tc.tile_pool(name="qk", bufs=2) as qkp,
            tc.tile_pool(name="vp", bufs=2) as vp,
            tc.tile_pool(name="ap", bufs=8) as ap_pool,
            tc.tile_pool(name="ep", bufs=3) as ep,
            tc.tile_pool(name="st", bufs=1, space="PSUM") as stp,
            tc.tile_pool(name="ot", bufs=2, space="PSUM") as otp,
        ):
            mask_sb = constp.tile([128, 256], av_dtype, tag="mask")

            # -- tiles for both pairs (bufs=2 pools keep both resident) -----
            tiles = []
            for pair in range(PAIRS_PER_CORE):
                qt_sb = qkp.tile([128, L], st_dtype, tag="qt")
                kt_sb = qkp.tile([128, L], st_dtype, tag="kt")
                v_sb = vp.tile([128, NT * VW], av_dtype, tag="v")
                tiles.append((qt_sb, kt_sb, v_sb))

            def v_slice(pair, si):
                return tiles[pair][2][:, si * VW : si * VW + E + 1]

            # -- PE warm-up junk tile (vector memset: gpsimd's scheduler
            # slot is taken by SWDGE issues which would delay the warm-up) --
            warm_sb = constp.tile([128, 640], st_dtype, tag="warm")
            nc.vector.memset(warm_sb, 0)

            # -- input loads, spread over the four DMA-capable engines ------
            # sync/scalar/vector are HWDGE (~625ns engine time per issue);
            # gpsimd is SWDGE (~1us per issue but its queue runs async).
            # First-use order: pair 0 banks go [1,2,3,0].
            def q_chunk(pair, ch, w=512):
                dst = tiles[pair][0]
                return (dst[:, w * ch : w * (ch + 1)], qt[pair][:, w * ch : w * (ch + 1)])

            def k_chunk(pair, ch, w=512):
                dst = tiles[pair][1]
                return (dst[:, w * ch : w * (ch + 1)], kt[pair][:, w * ch : w * (ch + 1)])

            def v_chunk(pair, ch, w=4 * VW):
                dst = tiles[pair][2]
                return (dst[:, w * ch : w * (ch + 1)], vv[pair][:, w * ch : w * (ch + 1)])

            for eng, (dst, src) in [
                (nc.sync, q_chunk(0, 2, 256)),  # qt cols [512:768]
                (nc.scalar, k_chunk(0, 0, 256)),  # kt tiles 0,1
                (nc.sync, q_chunk(0, 3, 256)),  # qt cols [768:1024]
                (nc.scalar, k_chunk(0, 1, 256)),  # kt tiles 2,3
            ]:
                eng.dma_start(out=dst, in_=src)

            # pre-trigger the exp ACT table load inside the DMA shadow so the
            # first real activation doesn't pay the ~1.3us table switch. The
            # dummy writes its OWN tile — it must NOT touch warm_sb, which
            # the PE warm-up reads (that dependency would chain the warm-up
            # behind scalar's DMA issues).
            tl_sb = constp.tile([128, 1], st_dtype, tag="tldummy")
            nc.scalar.activation(
                out=tl_sb[:], in_=warm_sb[:, 0:1], func=Exp, scale=1.0,
            )

            # The fill was DMA-queue-congestion-bound: each HWDGE queue moves
            # ~65KB/us, so spread the early transfers over the three queues
            # (sync / scalar / gpsimd-SWDGE) in NEED order: group 1 (first
            # diag) needs kt tiles 4-7 + mask by ~12.5us, AV(g0) needs v
            # tiles 0-3 by ~13.3us, v tiles 4-7 by ~14.6us.
            for eng, (dst, src) in [
                (nc.gpsimd, k_chunk(0, 1)),  # kt tiles 4..7 (group (1,1))
                (nc.gpsimd, v_chunk(0, 0)),  # AV(g0), ~13.3us
                (nc.scalar, (mask_sb[:], mask[:])),
                (nc.gpsimd, v_chunk(0, 1)),
                (nc.sync, q_chunk(0, 2)),  # bank-2 ST, ~13us
                (nc.sync, k_chunk(0, 2)),
                (nc.gpsimd, v_chunk(0, 2)),
                (nc.sync, q_chunk(0, 3)),
                (nc.sync, k_chunk(0, 3)),
                (nc.gpsimd, v_chunk(0, 3)),
                (nc.sync, q_chunk(0, 0)),
                # pair 1: first big chunks on gpsimd SWDGE + v on sync after
                # pair-0 (the rest are deferred into the group loop so the
                # gpsimd SWDGE issues spread out)
                (nc.gpsimd, q_chunk(1, 0, 1024)),
                (nc.gpsimd, k_chunk(1, 0, 1024)),
                (nc.sync, v_chunk(1, 0, 8 * VW)),
                (nc.sync, v_chunk(1, 1, 8 * VW)),
            ]:
                eng.dma_start(out=dst, in_=src)
            deferred_loads = [q_chunk(1, 1, 1024), k_chunk(1, 1, 1024)]

            # -- compute ---------------------------------------------------
            ot_banks = {}
            st_idx = [0]  # rotating tag counter for the 3 ST half-tiles

            def next_st():
                st = stp.tile([128, 1024], f32, tag=f"st{st_idx[0] % 3}", name="st")
                st_idx[0] += 1
                return st

            def emit_st_group(pair, lj, gi, split_first=False):
                qt_sb, kt_sb, v_sb = tiles[pair]
                a_grp = ap_pool.tile([128, 4 * 512], av_dtype, tag="A", name="A")
                a_i16 = a_grp[:].bitcast(i16)
                diag = gi == lj
                if not diag:
                    for hb in range(2):  # two [128,1024] half-tiles
                        st = next_st()
                        for cc in range(2):
                            w = 2 * hb + cc
                            si = 4 * gi + w
                            half = (w % 2) * E
                            # split_first: halve the moving cols so the very
                            # first group starts while its qt chunk is still
                            # landing
                            nsub = 2 if split_first else 1
                            for s2 in range(nsub):
                                cw = 512 // nsub
                                c0 = cw * s2
                                nc.tensor.matmul(
                                    st[:, 512 * cc + c0 : 512 * cc + c0 + cw],
                                    kt_sb[half : half + E, si * 128 : si * 128 + 128],
                                    qt_sb[half : half + E,
                                          512 * lj + c0 : 512 * lj + c0 + cw],
                                    start=True,
                                    stop=True,
                                )
                        if hb == 0:
                            nc.scalar.activation(
                                out=a_grp[:, 0:1024], in_=st[:, 0:1024],
                                func=Exp, scale=1.0,
                            )
                        else:
                            nc.vector.tensor_scalar(
                                a_i16[:, 1024:2048], st[:, 0:1024],
                                EXP_A, EXP_B,
                                mybir.AluOpType.mult, mybir.AluOpType.add,
                            )
                    return a_grp
                # Diagonal group: chunk c (s-tile 4gi+c) only needs l-cols
                # [128c:512]. PSUM allows one matmul accumulation region per
                # 512-col bank, so pack one chunk per bank:
                #   tile_hi bank0 [0:512]   = chunk 0 (full)
                #   tile_hi bank1 [512:640] = chunk 3 (128 cols)
                #   tile_lo bank0 [0:384]   = chunk 1
                #   tile_lo bank1 [512:768] = chunk 2
                # tile_hi goes FIRST through the fast DVE exp because AV
                # consumes chunk 0 first. a_grp cols mirror the tiles:
                # [0:768] <- tile_lo (ACT), [1024:1664] <- tile_hi (DVE),
                # putting all four 128-wide diagonal mask blocks at cols
                # 0/512/1024/1536 -> ONE strided multiply.
                st_lo = next_st()
                st_hi = next_st()
                si0 = 4 * gi
                hi_place = ((0, 0), (3, 512))
                lo_place = ((1, 0), (2, 512))
                for idx, (c, b0) in enumerate(hi_place):
                    si = si0 + c
                    half = (idx % 2) * E
                    nc.tensor.matmul(
                        st_hi[:, b0 : b0 + 512 - 128 * c],
                        kt_sb[half : half + E, si * 128 : si * 128 + 128],
                        qt_sb[half : half + E,
                              512 * lj + 128 * c : 512 * lj + 512],
                        start=True, stop=True,
                    )
                nc.vector.tensor_scalar(
                    a_i16[:, 1024:1664], st_hi[:, 0:640],
                    EXP_A, EXP_B, mybir.AluOpType.mult, mybir.AluOpType.add,
                )
                # mask the DVE-half diagonal blocks (cols 1024, 1536) right
                # away so AV chunk 0 isn't gated on the ACT half's exp
                blk = a_grp[:].rearrange("p (b c) -> p b c", c=128)
                msk = mask_sb[:].rearrange("p (b c) -> p b c", c=128)
                sel_hi = blk[:, 8:16:4, :]
                nc.vector.tensor_mul(sel_hi, sel_hi, msk[:, 0:2, :])
                for idx, (c, b0) in enumerate(lo_place):
                    si = si0 + c
                    half = (idx % 2) * E
                    nc.tensor.matmul(
                        st_lo[:, b0 : b0 + 512 - 128 * c],
                        kt_sb[half : half + E, si * 128 : si * 128 + 128],
                        qt_sb[half : half + E,
                              512 * lj + 128 * c : 512 * lj + 512],
                        start=True, stop=True,
                    )
                nc.scalar.activation(
                    out=a_grp[:, 0:768], in_=st_lo[:, 0:768],
                    func=Exp, scale=1.0,
                )
                sel_lo = blk[:, 0:5:4, :]  # cols 0, 512
                nc.vector.tensor_mul(sel_lo, sel_lo, msk[:, 0:2, :])
                return a_grp

            # a-col base of diag chunk c's valid [128c:512] suffix
            DIAG_A = {0: 1024, 1: 0, 2: 512, 3: 1536}

            def emit_av_group(pair, lj, gi, a_grp):
                ot = ot_banks[(pair, lj)]
                diag = gi == lj
                base = 512 * lj

                def av_chunk(c, skip=False):
                    si = 4 * gi + c
                    off = 128 * c if diag else 0
                    a0 = DIAG_A[c] if diag else 512 * c
                    nc.tensor.matmul(
                        ot[:, off:512],
                        v_slice(pair, si),
                        a_grp[:, a0 : a0 + 512 - off],
                        start=(gi == 0 and c == 0),
                        stop=(diag and c == 3),
                        skip_group_check=skip,
                    )

                if not diag:
                    for c in range(4):
                        av_chunk(c)
                    return
                # diagonal (= last) group of the bank: chunk 3 only touches
                # ot cols [384:512], so cols [0:384] are final after chunk 2
                # — evacuate them while chunk 3 still accumulates. (DMA can't
                # source PSUM; Copy shares the exp ACT table.) Non-final
                # banks use ONE [0:512] DMA (fewer ~700ns issue slots); the
                # final bank splits its DMA across sync+scalar so the two
                # issues overlap in the exec-time tail.
                final = (pair, lj) == (PAIRS_PER_CORE - 1, 0)
                for c in range(3):
                    av_chunk(c)
                ot_sb = ep.tile([E + 1, 512], f32, tag="osb", name="osb")
                if lj == 3:
                    # bank 3's evac runs right before the last bank's (bank
                    # 0) exp chain on the DVE — do it on ACT instead so the
                    # final group's TENSOR_SCALAR + mask aren't delayed
                    nc.scalar.activation(
                        out=ot_sb[:, 0:384], in_=ot[:, 0:384],
                        func=Copy, scale=1.0,
                    )
                else:
                    nc.vector.tensor_copy(ot_sb[:, 0:384], ot[:, 0:384])
                if final:
                    nc.sync.dma_start(
                        out=oo[pair][:, base : base + 384], in_=ot_sb[:, 0:384]
                    )
                av_chunk(3, skip=True)
                nc.scalar.activation(
                    out=ot_sb[:, 384:512], in_=ot[:, 384:512],
                    func=Copy, scale=1.0,
                )
                if final:
                    nc.scalar.dma_start(
                        out=oo[pair][:, base + 384 : base + 512],
                        in_=ot_sb[:, 384:512],
                    )
                else:
                    nc.sync.dma_start(
                        out=oo[pair][:, base : base + 512], in_=ot_sb[:]
                    )
                ot_banks.pop((pair, lj))

            # groups: (pair, lj, gi) — bank lj accumulates s-tiles 0..4lj+3
            # in groups of 4; gi == lj is the diagonal (partial) group.
            # Banks in order [1,2,3,0] so the last bank has a single AV group.
            groups = [
                (pair, lj, gi)
                for pair in range(PAIRS_PER_CORE)
                for lj in (1, 2, 3, 0)
                for gi in range(lj + 1)
            ]

            # PE p-state warm-up: run junk matmuls on a zeroed SBUF tile
            # while the first input DMAs are in flight so the Tensor clock
            # ramps toward 2.4GHz before the first real group.
            # sized to end roughly when the first input chunks land (too
            # long would DELAY the first real group: the PE is in-order)
            warm_st = stp.tile([128, 1024], f32, tag="st0", name="warm")
            for _ in range(6):
                for half in range(2):
                    p0 = 64 * half
                    nc.tensor.matmul(
                        warm_st[:, 512 * half : 512 * (half + 1)],
                        warm_sb[p0 : p0 + 64, 0:128],
                        warm_sb[p0 : p0 + 64, 128:640],
                        start=True,
                        stop=True,
                    )
            st_idx[0] = 1  # warm-up used tag st0; start real groups on st1

            prev = None
            Copy = mybir.ActivationFunctionType.Copy

            for g_idx, (pair, lj, gi) in enumerate(groups):
                if (pair, lj) not in ot_banks:
                    ot_banks[(pair, lj)] = otp.tile(
                        [E + 1, 512], f32, tag="ot", name="ot"
                    )
                if g_idx in (4, 6) and deferred_loads:
                    dst, src = deferred_loads.pop(0)
                    nc.gpsimd.dma_start(out=dst, in_=src)
                a_grp = emit_st_group(pair, lj, gi, split_first=(g_idx == 0))
                if prev is not None:
                    pp, plj, pgi, pa = prev
                    emit_av_group(pp, plj, pgi, pa)
                prev = (pair, lj, gi, a_grp)
            pp, plj, pgi, pa = prev
            emit_av_group(pp, plj, pgi, pa)

    return nc


# ---------------------------------------------------------------------------
# Host-side sharding / unsharding
# ---------------------------------------------------------------------------

def _in_maps(queries, keys, values, tau, delta, st_dtype=bf16, av_dtype=bf16):
    np_st = mybir.dt.np(st_dtype)
    np_av = mybir.dt.np(av_dtype)
    VW = E + 2
    NT = L // 128
    mask1 = np.triu(np.ones((128, 128), dtype=np.float32))
    mask = np.ascontiguousarray(np.tile(mask1, (1, 2))).astype(np_av)
    maps = []
    for c in range(N_CORES):
        ps = [2 * c, 2 * c + 1]
        b = ps[0] // H
        hs = [p % H for p in ps]
        qscale = np.float32(SCALE * tau[b, 0])
        # q/k transposed [E, L] and duplicated onto both partition halves
        qt = np.ascontiguousarray(
            np.stack([
                np.concatenate([queries[b, :, h, :].T * qscale] * 2, axis=0)
                for h in hs
            ])
        ).astype(np_st)
        kt = np.ascontiguousarray(
            np.stack([
                np.concatenate([keys[b, :, h, :].T] * 2, axis=0) for h in hs
            ])
        ).astype(np_st)
        # V augmented with the delta fold: cols 0..63 = V * exp(delta'),
        # col 64 = exp(delta') (denominator), col 65 pad. Laid out as
        # [128 (s%128), NT*VW] so each DMA chunk is contiguous per partition.
        expd = np.exp(SCALE * delta[b]).astype(np.float32)  # [L]
        vv = np.zeros((PAIRS_PER_CORE, L, VW), dtype=np.float32)
        for i, h in enumerate(hs):
            vv[i, :, 0:E] = values[b, :, h, :] * expd[:, None]
            vv[i, :, E] = expd
        vv = vv.reshape(PAIRS_PER_CORE, NT, 128, VW).transpose(0, 2, 1, 3)
        vv = np.ascontiguousarray(vv.reshape(PAIRS_PER_CORE, 128, NT * VW)).astype(np_av)
        maps.append({"qt": qt, "kt": kt, "vv": vv, "mask": mask})
    return maps


_CACHED = {}


def run(queries, keys, values, tau, delta, trace=False, st_dtype=bf16,
        av_dtype=bf16):
    key = (str(st_dtype), str(av_dtype))
    if key not in _CACHED:
        _CACHED[key] = build_program(st_dtype, av_dtype)
    nc = _CACHED[key]
    in_maps = _in_maps(
        np.asarray(queries),
        np.asarray(keys),
        np.asarray(values),
        np.asarray(tau),
        np.asarray(delta),
        st_dtype=st_dtype,
        av_dtype=av_dtype,
    )
    res = run_bass_kernel_spmd(
        nc, in_maps, core_ids=list(range(N_CORES)), trace=trace
    )
    out = np.empty((B, L, H, E), dtype=np.float32)
    for c in range(N_CORES):
        o = np.asarray(res.results[c]["oo"], dtype=np.float32)  # [2, 65, L]
        for i, p in enumerate([2 * c, 2 * c + 1]):
            out[p // H, :, p % H, :] = (o[i, 0:E, :] / o[i, E : E + 1, :]).T
    return out, res


def kernel(queries, keys, values, tau, delta):
    out, _ = run(queries, keys, values, tau, delta, trace=False)
    return out
